# revision 9
# baseline (speedup 1.0000x reference)
"""Trainium2 Bass kernel for nn_Memory_6554120093948 (scatter_memory).

Contract: kernel(**inputs) takes FULL unsharded numpy inputs and returns the
FULL output tuple (center_new, source_mem_new, class2center_new, interdist_new).

Strategy (8 NeuronCores, batch-sharded):
  - shard feature_source/label_source/index_source over batch (2048 rows/core)
  - replicate center_mem; compute per-core partial segment-sums + the
    per-class distance dot partials with one f32r matmul; AllReduce them
  - each core computes center_new, normalizes, and runs the second cosine
    distance matmul; top-2 smallest distances extracted with exact-fp32
    max / masked-max reductions (indices via iota dot)
  - momentum rows for the core's index range updated on device
  - host assembles: full source_mem copy + row scatter, interdist histogram
    (np.add.at over the 16K (top1,top2) pairs), class2center counts
"""

import os
import numpy as np

import concourse.bass as bass
import concourse.bacc as bacc
import concourse.mybir as mybir
import concourse.tile as tile
from concourse.bass_utils import run_bass_kernel_spmd


def _install_ntff_hook():
    """Best-effort: wire the axon NTFF profiling hook into antenv so
    run_bass_kernel_spmd(trace=True) can capture exec times under axon."""
    try:
        import sys
        import types
        import antenv
        try:
            from antenv.axon_hooks import get_axon_ntff_profile_hook  # noqa: F401
            return  # already present
        except ImportError:
            pass
        from trn_agent_boot.trn_boot import _ntff_profile_via_ctypes
        hook = _ntff_profile_via_ctypes("/opt/axon/libaxon_pjrt.so")
        mod = types.ModuleType("antenv.axon_hooks")
        _holder = [hook]
        mod.set_axon_ntff_profile_hook = lambda h: _holder.__setitem__(0, h)
        mod.get_axon_ntff_profile_hook = lambda: _holder[0]
        sys.modules["antenv.axon_hooks"] = mod
        antenv.axon_hooks = mod
    except Exception:
        pass


_install_ntff_hook()

dt = mybir.dt
AF = mybir.ActivationFunctionType
ALU = mybir.AluOpType
AX = mybir.AxisListType

C = 1000
D = 256
MOMENTUM = 0.1
N_CORES = 8

MT = 8                      # center row tiles
PM = [min(128, C - m * 128) for m in range(MT)]
KD = 2                      # D // 128
NSL = [(0, 512), (512, C)]  # d2 free-dim slices (psum banks)

# idx extraction engine: 'gpsimd' or 'vector'
IDX_ENGINE = os.environ.get("KRN_IDX_ENGINE", "vector")


def build_program(b_local, n_cores=N_CORES):
    T = b_local // 128
    nc = bacc.Bacc("TRN2", target_bir_lowering=False, debug=False,
                   num_devices=n_cores)

    feat_d = nc.dram_tensor("feat", [b_local, D], dt.float32, kind="ExternalInput").ap()
    cenm_d = nc.dram_tensor("cenm", [C, D], dt.float32, kind="ExternalInput").ap()
    labf_d = nc.dram_tensor("labf", [128, T], dt.float32, kind="ExternalInput").ap()
    old_d = nc.dram_tensor("oldrows", [b_local, D], dt.float32, kind="ExternalInput").ap()

    o_center = nc.dram_tensor("o_center", [C, D], dt.float32, kind="ExternalOutput").ap()
    o_newrows = nc.dram_tensor("o_newrows", [b_local, D], dt.float32, kind="ExternalOutput").ap()
    o_cdist = nc.dram_tensor("o_cdist", [C, 1], dt.float32, kind="ExternalOutput").ap()
    o_top1 = nc.dram_tensor("o_top1", [128, T], dt.int32, kind="ExternalOutput").ap()
    o_top2 = nc.dram_tensor("o_top2", [128, T], dt.int32, kind="ExternalOutput").ap()
    o_inter = nc.dram_tensor("o_inter", [128, T], dt.float32, kind="ExternalOutput").ap()

    idx_eng = nc.gpsimd if IDX_ENGINE == "gpsimd" else nc.vector

    with tile.TileContext(nc) as tc:
        with tc.tile_pool(name="const", bufs=1) as cpool, \
             tc.tile_pool(name="big", bufs=1) as bpool, \
             tc.tile_pool(name="work", bufs=3) as wpool, \
             tc.tile_pool(name="tiny", bufs=4) as tpool, \
             tc.tile_pool(name="dram", bufs=1, space="DRAM") as dpool:

            # ---------- constants ----------
            iota_rev = cpool.tile([128, C], dt.float32)
            nc.gpsimd.iota(iota_rev[:], pattern=[[-1, C]], base=C - 1,
                           channel_multiplier=0, allow_small_or_imprecise_dtypes=True)
            iota_c = cpool.tile([128, C], dt.float32)
            nc.gpsimd.iota(iota_c[:], pattern=[[1, C]], base=0,
                           channel_multiplier=0, allow_small_or_imprecise_dtypes=True)
            ident = cpool.tile([128, 128], dt.float32)
            idio = cpool.tile([128, 128], dt.int32)
            nc.gpsimd.iota(idio[:], pattern=[[1, 128]], base=0, channel_multiplier=-1)
            nc.vector.tensor_scalar(ident[:], idio[:], 0, None, ALU.is_equal)
            ones_t = cpool.tile([128, 512], dt.float32)
            nc.vector.memset(ones_t[:], 1.0)

            labf = cpool.tile([128, T], dt.float32)
            nc.sync.dma_start(labf[:], labf_d[:])

            # ---------- load + normalize features ----------
            rh = [bpool.tile([128, 2 * D], dt.float32, tag=f"rh{t}", name=f"rh{t}")
                  for t in range(T)]
            old = [bpool.tile([128, D], dt.float32, tag=f"old{t}", name=f"old{t}")
                   for t in range(T)]
            for t in range(T):
                nc.sync.dma_start(rh[t][:, 0:D], feat_d[t * 128:(t + 1) * 128, :])
                nc.sync.dma_start(old[t][:], old_d[t * 128:(t + 1) * 128, :])

            for t in range(T):
                sq = wpool.tile([128, D], dt.float32, tag="sq", name="sq")
                ss = tpool.tile([128, 1], dt.float32, tag="ss", name="ss")
                nc.vector.scalar_tensor_tensor(sq[:], rh[t][:, 0:D], 1.0,
                                               rh[t][:, 0:D], ALU.bypass, ALU.mult,
                                               accum_out=ss[:])
                nrm = tpool.tile([128, 1], dt.float32, tag="nrm", name="nrm")
                nc.scalar.sqrt(nrm[:], ss[:])
                rin = tpool.tile([128, 1], dt.float32, tag="rin", name="rin")
                nc.vector.reciprocal(rin[:], nrm[:])
                nc.scalar.mul(rh[t][:, D:2 * D], rh[t][:, 0:D], rin[:])

            # ---------- momentum update (independent; fills AR gap) ----------
            for t in range(T):
                scrm = wpool.tile([128, D], dt.float32, tag="scrm", name="scrm")
                nc.vector.scalar_tensor_tensor(scrm[:], old[t][:], 9.0, rh[t][:, 0:D],
                                               ALU.mult, ALU.add)
                nc.scalar.mul(old[t][:], scrm[:], MOMENTUM)
                nc.sync.dma_start(o_newrows[t * 128:(t + 1) * 128, :], old[t][:])

            # ---------- f-hat transposes ----------
            fhT = [bpool.tile([128, b_local], dt.float32, tag=f"fhT{k}", name=f"fhT{k}")
                   for k in range(KD)]
            with tc.tile_pool(name="ptr", bufs=2, space="PSUM") as ptr:
                for t in range(T):
                    for k in range(KD):
                        pt = ptr.tile([128, 128], dt.float32, tag="tr", name="tr")
                        nc.tensor.transpose(pt[:], rh[t][:, D + k * 128:D + (k + 1) * 128],
                                            ident[:])
                        nc.scalar.copy(fhT[k][:, t * 128:(t + 1) * 128], pt[:])

            # ---------- load + normalize centers ----------
            cm = [bpool.tile([128, D], dt.float32, tag=f"cm{m}", name=f"cm{m}")
                  for m in range(MT)]
            chat = [bpool.tile([128, D], dt.float32, tag=f"chat{m}", name=f"chat{m}")
                    for m in range(MT)]
            for m in range(MT):
                pm = PM[m]
                nc.sync.dma_start(cm[m][:pm], cenm_d[m * 128:m * 128 + pm, :])
                sq = wpool.tile([128, D], dt.float32, tag="sq", name="sq")
                ss = tpool.tile([128, 1], dt.float32, tag="ss", name="ss")
                nc.vector.scalar_tensor_tensor(sq[:pm], cm[m][:pm], 1.0, cm[m][:pm],
                                               ALU.bypass, ALU.mult, accum_out=ss[:pm])
                nrm = tpool.tile([128, 1], dt.float32, tag="nrm", name="nrm")
                nc.scalar.sqrt(nrm[:pm], ss[:pm])
                rin = tpool.tile([128, 1], dt.float32, tag="rin", name="rin")
                nc.vector.reciprocal(rin[:pm], nrm[:pm])
                nc.scalar.mul(chat[m][:pm], cm[m][:pm], rin[:pm])

            # ---------- segment-sum matmul:  out[C, 512] = onehot^T @ [f | fhat] ----------
            ar_in = dpool.tile([C, D + 1], dt.float32)
            with tc.tile_pool(name="pseg", bufs=1, space="PSUM") as pseg:
                psum_seg = [pseg.tile([128, 512], dt.float32, tag=f"seg{m}", name=f"seg{m}")
                            for m in range(MT)]
                for t in range(T):
                    oh = wpool.tile([128, C], dt.float32, tag="oh", name="oh")
                    nc.gpsimd.tensor_scalar(oh[:], iota_c[:], labf[:, t:t + 1], None,
                                            ALU.is_equal)
                    for m in range(MT):
                        nc.tensor.matmul(psum_seg[m][:PM[m], :],
                                         oh[:, m * 128:m * 128 + PM[m]],
                                         rh[t][:],
                                         start=(t == 0), stop=(t == T - 1))

                # classdist partials + AR pack
                for m in range(MT):
                    pm = PM[m]
                    sq = wpool.tile([128, D], dt.float32, tag="sq", name="sq")
                    dotc = tpool.tile([128, 1], dt.float32, tag="dotc", name="dotc")
                    nc.vector.scalar_tensor_tensor(sq[:pm], psum_seg[m][:pm, D:2 * D],
                                                   1.0, chat[m][:pm], ALU.bypass,
                                                   ALU.mult, accum_out=dotc[:pm])
                    cdp = tpool.tile([128, 1], dt.float32, tag="cdp", name="cdp")
                    nc.vector.tensor_scalar(cdp[:pm], dotc[:pm], -0.5, None, ALU.mult)
                    segsb = wpool.tile([128, D], dt.float32, tag="segsb", name="segsb")
                    nc.scalar.copy(segsb[:pm], psum_seg[m][:pm, 0:D])
                    nc.sync.dma_start(ar_in[m * 128:m * 128 + pm, 0:D], segsb[:pm])
                    nc.sync.dma_start(ar_in[m * 128:m * 128 + pm, D:D + 1], cdp[:pm])

            # ---------- AllReduce ----------
            ar_out = dpool.tile([C, D + 1], dt.float32,
                                addr_space="Shared" if n_cores > 4 else "Local")
            nc.gpsimd.collective_compute(
                "AllReduce", ALU.add,
                replica_groups=[list(range(n_cores))],
                ins=[ar_in.opt()], outs=[ar_out.opt()],
            )
            nc.sync.dma_start(o_cdist[:], ar_out[:, D:D + 1])

            # ---------- center_new + normalize ----------
            chn = [bpool.tile([128, D], dt.float32, tag=f"chn{m}", name=f"chn{m}")
                   for m in range(MT)]
            for m in range(MT):
                pm = PM[m]
                seg = wpool.tile([128, D], dt.float32, tag="seg", name="seg")
                nc.sync.dma_start(seg[:pm], ar_out[m * 128:m * 128 + pm, 0:D])
                cn = wpool.tile([128, D], dt.float32, tag="cn", name="cn")
                nc.vector.tensor_add(cn[:pm], cm[m][:pm], seg[:pm])
                nc.sync.dma_start(o_center[m * 128:m * 128 + pm, :], cn[:pm])
                sq = wpool.tile([128, D], dt.float32, tag="sq", name="sq")
                ss = tpool.tile([128, 1], dt.float32, tag="ss", name="ss")
                nc.vector.scalar_tensor_tensor(sq[:pm], cn[:pm], 1.0, cn[:pm],
                                               ALU.bypass, ALU.mult, accum_out=ss[:pm])
                nrm = tpool.tile([128, 1], dt.float32, tag="nrm", name="nrm")
                nc.scalar.sqrt(nrm[:pm], ss[:pm])
                rin = tpool.tile([128, 1], dt.float32, tag="rin", name="rin")
                nc.vector.reciprocal(rin[:pm], nrm[:pm])
                nc.scalar.mul(chn[m][:pm], cn[:pm], rin[:pm])

            # ---------- c-hat-new transposes ----------
            cT = [bpool.tile([128, C], dt.float32, tag=f"cT{k}", name=f"cT{k}")
                  for k in range(KD)]
            with tc.tile_pool(name="ptr2", bufs=2, space="PSUM") as ptr2:
                for m in range(MT):
                    pm = PM[m]
                    for k in range(KD):
                        pt = ptr2.tile([128, 128], dt.float32, tag="tr2", name="tr2")
                        nc.tensor.transpose(pt[:, :pm], chn[m][:pm, k * 128:(k + 1) * 128],
                                            ident[:pm, :pm])
                        nc.scalar.copy(cT[k][:, m * 128:m * 128 + pm], pt[:, :pm])

            # ---------- d2 matmul + exact top-2 ----------
            t1buf = cpool.tile([128, T], dt.int32)
            t2buf = cpool.tile([128, T], dt.int32)
            inbuf = cpool.tile([128, T], dt.float32)
            with tc.tile_pool(name="pd2", bufs=2, space="PSUM") as pd2:
                for t in range(T):
                    sp1 = pd2.tile([128, C], dt.float32, tag="sp1", name="sp1")
                    for (n0, n1) in NSL:
                        for k in range(KD):
                            nc.tensor.matmul(sp1[:, n0:n1],
                                             fhT[k][:, t * 128:(t + 1) * 128],
                                             cT[k][:, n0:n1],
                                             start=(k == 0), stop=False)
                        # +1 everywhere → sp1 = s + 1 >= 0
                        nc.tensor.matmul(sp1[:, n0:n1],
                                         ones_t[0:1, 0:128],
                                         ones_t[0:1, 0:n1 - n0],
                                         start=False, stop=True)

                    m1p = tpool.tile([128, 1], dt.float32, tag="m1p", name="m1p")
                    nc.vector.tensor_reduce(m1p[:], sp1[:], AX.X, ALU.max)
                    ssb = wpool.tile([128, C], dt.float32, tag="ssb", name="ssb")
                    nc.scalar.copy(ssb[:], sp1[:])
                    masked = wpool.tile([128, C], dt.float32, tag="masked", name="masked")
                    nc.vector.scalar_tensor_tensor(masked[:], ssb[:], m1p[:], ssb[:],
                                                   ALU.is_lt, ALU.mult)
                    m2p = tpool.tile([128, 1], dt.float32, tag="m2p", name="m2p")
                    nc.vector.tensor_reduce(m2p[:], masked[:], AX.X, ALU.max)

                    scr1 = wpool.tile([128, C], dt.float32, tag="scr1", name="scr1")
                    idx1 = tpool.tile([128, 1], dt.float32, tag="idx1", name="idx1")
                    idx_eng.scalar_tensor_tensor(scr1[:], masked[:], 0.0, iota_rev[:],
                                                 ALU.is_le, ALU.mult, accum_out=idx1[:])
                    scr2 = wpool.tile([128, C], dt.float32, tag="scr2", name="scr2")
                    idx2 = tpool.tile([128, 1], dt.float32, tag="idx2", name="idx2")
                    idx_eng.scalar_tensor_tensor(scr2[:], masked[:], m2p[:], iota_rev[:],
                                                 ALU.is_ge, ALU.mult, accum_out=idx2[:])

                    # decode: top = 999 - idx ; inter = 0.5*(m1p - m2p)
                    nc.vector.tensor_scalar(t1buf[:, t:t + 1], idx1[:], -1.0, float(C - 1),
                                            ALU.mult, ALU.add)
                    nc.vector.tensor_scalar(t2buf[:, t:t + 1], idx2[:], -1.0, float(C - 1),
                                            ALU.mult, ALU.add)
                    dd = tpool.tile([128, 1], dt.float32, tag="dd", name="dd")
                    nc.vector.tensor_sub(dd[:], m1p[:], m2p[:])
                    nc.vector.tensor_scalar(inbuf[:, t:t + 1], dd[:], 0.5, None, ALU.mult)

            nc.sync.dma_start(o_top1[:], t1buf[:])
            nc.sync.dma_start(o_top2[:], t2buf[:])
            nc.sync.dma_start(o_inter[:], inbuf[:])

    nc.compile()
    return nc


_prog_cache = {}


def _get_program(b_local, n_cores):
    key = (b_local, n_cores)
    if key not in _prog_cache:
        _prog_cache[key] = build_program(b_local, n_cores)
    return _prog_cache[key]


LAST_RESULTS = None  # BassKernelResults of the last run (for test harness)


def kernel(feature_source, center_mem, source_mem, class2center, interdist,
           label_source, index_source, n_cores=N_CORES, trace=False,
           _runner=None):
    feature_source = np.asarray(feature_source, dtype=np.float32)
    center_mem = np.asarray(center_mem, dtype=np.float32)
    source_mem = np.asarray(source_mem, dtype=np.float32)
    class2center = np.asarray(class2center, dtype=np.float32)
    interdist = np.asarray(interdist, dtype=np.float32)
    label_source = np.asarray(label_source, dtype=np.int32)
    index_source = np.asarray(index_source, dtype=np.int32)

    B = feature_source.shape[0]
    b_local = B // n_cores
    T = b_local // 128

    nc = _get_program(b_local, n_cores)

    in_maps = []
    for k in range(n_cores):
        sl = slice(k * b_local, (k + 1) * b_local)
        lab_k = label_source[sl].astype(np.float32).reshape(T, 128).T.copy()
        in_maps.append({
            "feat": feature_source[sl],
            "cenm": center_mem,
            "labf": lab_k,
            "oldrows": np.ascontiguousarray(source_mem[index_source[sl]]),
        })

    global LAST_RESULTS
    if _runner is not None:
        outs = _runner(nc, in_maps)
    else:
        res = run_bass_kernel_spmd(nc, in_maps, core_ids=list(range(n_cores)),
                                   trace=trace,
                                   trace_cores=list(range(n_cores)) if trace else None)
        LAST_RESULTS = res
        outs = res.results

    # ---------- host assembly ----------
    center_new = outs[0]["o_center"]

    source_mem_new = source_mem.copy()
    new_rows = np.concatenate(
        [outs[k]["o_newrows"] for k in range(n_cores)], axis=0)
    source_mem_new[index_source] = new_rows

    counts = np.bincount(label_source, minlength=C).astype(np.float32)
    class2center_new = class2center.copy()
    class2center_new[:, 0] += counts
    class2center_new[:, 1] += 0.5 * counts + outs[0]["o_cdist"][:, 0]

    top1 = np.concatenate(
        [outs[k]["o_top1"].T.reshape(-1) for k in range(n_cores)])
    top2 = np.concatenate(
        [outs[k]["o_top2"].T.reshape(-1) for k in range(n_cores)])
    inter = np.concatenate(
        [outs[k]["o_inter"].T.reshape(-1) for k in range(n_cores)])

    interdist_new = interdist.copy()
    flat = interdist_new.reshape(C * C, 2)
    pair = top1.astype(np.int64) * C + top2.astype(np.int64)
    np.add.at(flat[:, 0], pair, 1.0)
    np.add.at(flat[:, 1], pair, inter.astype(np.float64))

    return (center_new, source_mem_new, class2center_new, interdist_new)


# revision 21
# speedup vs baseline: 1.8080x; 1.8080x over previous
"""Trainium2 Bass kernel for nn_Memory_6554120093948 (scatter_memory).

Contract: kernel(**inputs) takes FULL unsharded numpy inputs and returns the
FULL output tuple (center_new, source_mem_new, class2center_new, interdist_new).

Strategy (8 NeuronCores, batch-sharded):
  - shard feature_source/label_source/index_source over batch (2048 rows/core)
  - replicate center_mem; compute per-core partial segment-sums + the
    per-class distance dot partials with one f32r matmul; AllReduce them
  - each core computes center_new, normalizes, and runs the second cosine
    distance matmul; top-2 smallest distances extracted with exact-fp32
    max / masked-max reductions (indices via iota dot)
  - momentum rows for the core's index range updated on device
  - host assembles: full source_mem copy + row scatter, interdist histogram
    (np.add.at over the 16K (top1,top2) pairs), class2center counts
"""

import os
import numpy as np

import concourse.bass as bass
import concourse.bacc as bacc
import concourse.mybir as mybir
import concourse.tile as tile
from concourse.bass_utils import run_bass_kernel_spmd


def _install_ntff_hook():
    """Best-effort: wire the axon NTFF profiling hook into antenv so
    run_bass_kernel_spmd(trace=True) can capture exec times under axon."""
    try:
        import sys
        import types
        import antenv
        try:
            from antenv.axon_hooks import get_axon_ntff_profile_hook  # noqa: F401
            return  # already present
        except ImportError:
            pass
        from trn_agent_boot.trn_boot import _ntff_profile_via_ctypes
        hook = _ntff_profile_via_ctypes("/opt/axon/libaxon_pjrt.so")
        mod = types.ModuleType("antenv.axon_hooks")
        _holder = [hook]
        mod.set_axon_ntff_profile_hook = lambda h: _holder.__setitem__(0, h)
        mod.get_axon_ntff_profile_hook = lambda: _holder[0]
        sys.modules["antenv.axon_hooks"] = mod
        antenv.axon_hooks = mod
    except Exception:
        pass


_install_ntff_hook()

dt = mybir.dt
AF = mybir.ActivationFunctionType
ALU = mybir.AluOpType
AX = mybir.AxisListType

C = 1000
D = 256
MOMENTUM = 0.1
N_CORES = 8

MT = 8                      # center row tiles
PM = [min(128, C - m * 128) for m in range(MT)]
KD = 2                      # D // 128
NSL = [(0, 512), (512, C)]  # d2 free-dim slices (psum banks)

# idx extraction engine: 'gpsimd' or 'vector'
IDX_ENGINE = os.environ.get("KRN_IDX_ENGINE", "vector")


def build_program(b_local, n_cores=N_CORES):
    T = b_local // 128
    nc = bacc.Bacc("TRN2", target_bir_lowering=False, debug=False,
                   num_devices=n_cores)

    feat_d = nc.dram_tensor("feat", [b_local, D], dt.float32, kind="ExternalInput").ap()
    cenm_d = nc.dram_tensor("cenm", [C, D], dt.float32, kind="ExternalInput").ap()
    labf_d = nc.dram_tensor("labf", [128, T], dt.float32, kind="ExternalInput").ap()
    old_d = nc.dram_tensor("oldrows", [b_local, D], dt.float32, kind="ExternalInput").ap()

    o_center = nc.dram_tensor("o_center", [C, D], dt.float32, kind="ExternalOutput").ap()
    o_newrows = nc.dram_tensor("o_newrows", [b_local, D], dt.float32, kind="ExternalOutput").ap()
    o_cdist = nc.dram_tensor("o_cdist", [C, 1], dt.float32, kind="ExternalOutput").ap()
    o_i1 = nc.dram_tensor("o_i1", [128, T], dt.float32, kind="ExternalOutput").ap()
    o_is = nc.dram_tensor("o_is", [128, T], dt.float32, kind="ExternalOutput").ap()
    o_m1 = nc.dram_tensor("o_m1", [128, T], dt.float32, kind="ExternalOutput").ap()
    o_m2 = nc.dram_tensor("o_m2", [128, T], dt.float32, kind="ExternalOutput").ap()

    with tile.TileContext(nc) as tc:
        with tc.tile_pool(name="const", bufs=1) as cpool, \
             tc.tile_pool(name="big", bufs=1) as bpool, \
             tc.tile_pool(name="work", bufs=3) as wpool, \
             tc.tile_pool(name="tiny", bufs=4) as tpool, \
             tc.tile_pool(name="dram", bufs=1, space="DRAM") as dpool:

            # ---------- constants ----------
            iota_rev = cpool.tile([128, C], dt.float32)
            nc.gpsimd.iota(iota_rev[:], pattern=[[-1, C]], base=C - 1,
                           channel_multiplier=0, allow_small_or_imprecise_dtypes=True)
            iota_c = cpool.tile([128, C], dt.float32)
            nc.gpsimd.iota(iota_c[:], pattern=[[1, C]], base=0,
                           channel_multiplier=0, allow_small_or_imprecise_dtypes=True)
            ident = cpool.tile([128, 128], dt.float32)
            idio = cpool.tile([128, 128], dt.int32)
            nc.gpsimd.iota(idio[:], pattern=[[1, 128]], base=0, channel_multiplier=-1)
            nc.vector.tensor_scalar(ident[:], idio[:], 0, None, ALU.is_equal)
            labf = cpool.tile([128, T], dt.float32)
            nc.sync.dma_start(labf[:], labf_d[:])

            # ---------- load + normalize features ----------
            rh = [bpool.tile([128, 2 * D], dt.float32, tag=f"rh{t}", name=f"rh{t}")
                  for t in range(T)]
            old = [bpool.tile([128, D], dt.float32, tag=f"old{t}", name=f"old{t}")
                   for t in range(T)]
            for t in range(T):
                nc.sync.dma_start(rh[t][:, 0:D], feat_d[t * 128:(t + 1) * 128, :])
                nc.sync.dma_start(old[t][:], old_d[t * 128:(t + 1) * 128, :])

            for t in range(T):
                sq = wpool.tile([128, D], dt.float32, tag="sq", name="sq")
                ss = tpool.tile([128, 1], dt.float32, tag="ss", name="ss")
                nc.vector.scalar_tensor_tensor(sq[:], rh[t][:, 0:D], 1.0,
                                               rh[t][:, 0:D], ALU.bypass, ALU.mult,
                                               accum_out=ss[:])
                nrm = tpool.tile([128, 1], dt.float32, tag="nrm", name="nrm")
                nc.scalar.sqrt(nrm[:], ss[:])
                rin = tpool.tile([128, 1], dt.float32, tag="rin", name="rin")
                nc.vector.reciprocal(rin[:], nrm[:])
                nc.scalar.mul(rh[t][:, D:2 * D], rh[t][:, 0:D], rin[:])

            # ---------- persistent exact bf16 3-way split of [f | fhat] ----------
            spl_a = [bpool.tile([128, 2 * D], dt.bfloat16, tag=f"sa{t}", name=f"sa{t}")
                     for t in range(T)]
            spl_b = [bpool.tile([128, 2 * D], dt.bfloat16, tag=f"sb{t}", name=f"sb{t}")
                     for t in range(T)]
            spl_c = [bpool.tile([128, D], dt.bfloat16, tag=f"sc{t}", name=f"sc{t}")
                     for t in range(T)]
            for t in range(T):
                nc.scalar.copy(spl_a[t][:], rh[t][:])
                r1 = wpool.tile([128, 2 * D], dt.float32, tag="r1s", name="r1s", bufs=2)
                nc.vector.tensor_sub(r1[:], rh[t][:], spl_a[t][:])
                nc.scalar.copy(spl_b[t][:], r1[:])
                r2 = wpool.tile([128, D], dt.float32, tag="r2s", name="r2s", bufs=2)
                nc.vector.tensor_sub(r2[:], r1[:, 0:D], spl_b[t][:, 0:D])
                nc.scalar.copy(spl_c[t][:], r2[:])

            # ---------- load + normalize centers ----------
            cm = [bpool.tile([128, D], dt.float32, tag=f"cm{m}", name=f"cm{m}")
                  for m in range(MT)]
            chat = [bpool.tile([128, D], dt.float32, tag=f"chat{m}", name=f"chat{m}")
                    for m in range(MT)]
            for m in range(MT):
                pm = PM[m]
                nc.sync.dma_start(cm[m][:pm], cenm_d[m * 128:m * 128 + pm, :])
                sq = wpool.tile([128, D], dt.float32, tag="sq", name="sq")
                ss = tpool.tile([128, 1], dt.float32, tag="ss", name="ss")
                nc.vector.scalar_tensor_tensor(sq[:pm], cm[m][:pm], 1.0, cm[m][:pm],
                                               ALU.bypass, ALU.mult, accum_out=ss[:pm])
                nrm = tpool.tile([128, 1], dt.float32, tag="nrm", name="nrm")
                nc.scalar.sqrt(nrm[:pm], ss[:pm])
                rin = tpool.tile([128, 1], dt.float32, tag="rin", name="rin")
                nc.vector.reciprocal(rin[:pm], nrm[:pm])
                nc.scalar.mul(chat[m][:pm], cm[m][:pm], rin[:pm])

            # ---------- segment-sum matmuls (one sweep, 8 psum banks) ----------
            ar_in = dpool.tile([C, D + 1], dt.float32)
            with tc.tile_pool(name="pseg", bufs=1, space="PSUM") as pseg:
                psum_seg = [pseg.tile([128, 512], dt.float32, tag=f"seg{m}", name=f"seg{m}")
                            for m in range(MT)]
                for t in range(T):
                    oh = wpool.tile([128, C], dt.bfloat16, tag="oh", name="oh")
                    nc.vector.tensor_scalar(oh[:], iota_c[:], labf[:, t:t + 1], None,
                                            ALU.is_equal)
                    for m in range(MT):
                        ohs = oh[:, m * 128:m * 128 + PM[m]]
                        ps = psum_seg[m]
                        nc.tensor.matmul(ps[:PM[m], :], ohs, spl_a[t][:],
                                         start=(t == 0), stop=False)
                        nc.tensor.matmul(ps[:PM[m], :], ohs, spl_b[t][:],
                                         start=False, stop=False)
                        nc.tensor.matmul(ps[:PM[m], 0:D], ohs, spl_c[t][:],
                                         start=False, stop=(t == T - 1))

                for m in range(MT):
                    pm = PM[m]
                    ps = psum_seg[m]
                    sq = wpool.tile([128, D], dt.float32, tag="sq", name="sq")
                    dotc = tpool.tile([128, 1], dt.float32, tag="dotc", name="dotc")
                    nc.vector.scalar_tensor_tensor(sq[:pm], ps[:pm, D:2 * D],
                                                   1.0, chat[m][:pm], ALU.bypass,
                                                   ALU.mult, accum_out=dotc[:pm])
                    cdp = tpool.tile([128, 1], dt.float32, tag="cdp", name="cdp")
                    nc.vector.tensor_scalar(cdp[:pm], dotc[:pm], -0.5, None, ALU.mult)
                    segsb = wpool.tile([128, D], dt.float32, tag="segsb", name="segsb")
                    nc.scalar.copy(segsb[:pm], ps[:pm, 0:D])
                    nc.sync.dma_start(ar_in[m * 128:m * 128 + pm, 0:D], segsb[:pm])
                    nc.sync.dma_start(ar_in[m * 128:m * 128 + pm, D:D + 1], cdp[:pm])

            # ---------- AllReduce ----------
            ar_out = dpool.tile([C, D + 1], dt.float32,
                                addr_space="Shared" if n_cores > 4 else "Local")
            nc.gpsimd.collective_compute(
                "AllReduce", ALU.add,
                replica_groups=[list(range(n_cores))],
                ins=[ar_in.opt()], outs=[ar_out.opt()],
            )
            nc.sync.dma_start(o_cdist[:], ar_out[:, D:D + 1])

            # ---------- gap fillers (scheduled during the AllReduce) ----------
            fhT = [bpool.tile([128, b_local], dt.float32, tag=f"fhT{k}", name=f"fhT{k}")
                   for k in range(KD)]
            with tc.tile_pool(name="ptr", bufs=2, space="PSUM") as ptr:
                for t in range(T):
                    for k in range(KD):
                        pt = ptr.tile([128, 128], dt.float32, tag="tr", name="tr")
                        nc.tensor.transpose(pt[:], rh[t][:, D + k * 128:D + (k + 1) * 128],
                                            ident[:])
                        nc.scalar.copy(fhT[k][:, t * 128:(t + 1) * 128], pt[:])

            for t in range(T):
                scrm = wpool.tile([128, D], dt.float32, tag="scrm", name="scrm")
                nc.vector.scalar_tensor_tensor(scrm[:], old[t][:], 9.0, rh[t][:, 0:D],
                                               ALU.mult, ALU.add)
                nc.scalar.mul(old[t][:], scrm[:], MOMENTUM)
                nc.gpsimd.dma_start(o_newrows[t * 128:(t + 1) * 128, :], old[t][:])

            # ---------- center_new + normalize + transpose ----------
            chn = [bpool.tile([128, D], dt.float32, tag=f"chn{m}", name=f"chn{m}")
                   for m in range(MT)]
            cT = [bpool.tile([128, C], dt.float32, tag=f"cT{k}", name=f"cT{k}")
                  for k in range(KD)]
            segs = [wpool.tile([128, D], dt.float32, tag=f"segm{m}",
                                name=f"segm{m}", bufs=1) for m in range(MT)]
            for m in range(MT):
                nc.sync.dma_start(segs[m][:PM[m]], ar_out[m * 128:m * 128 + PM[m], 0:D])
            with tc.tile_pool(name="ptr2", bufs=2, space="PSUM") as ptr2:
                for m in range(MT):
                    pm = PM[m]
                    seg = segs[m]
                    cn = wpool.tile([128, D], dt.float32, tag="cn", name="cn", bufs=4)
                    nc.vector.tensor_add(cn[:pm], cm[m][:pm], seg[:pm])
                    nc.gpsimd.dma_start(o_center[m * 128:m * 128 + pm, :], cn[:pm])
                    sq = wpool.tile([128, D], dt.float32, tag="sq", name="sq")
                    ss = tpool.tile([128, 1], dt.float32, tag="ss", name="ss")
                    nc.vector.scalar_tensor_tensor(sq[:pm], cn[:pm], 1.0, cn[:pm],
                                                   ALU.bypass, ALU.mult,
                                                   accum_out=ss[:pm])
                    nrm = tpool.tile([128, 1], dt.float32, tag="nrm", name="nrm")
                    nc.scalar.sqrt(nrm[:pm], ss[:pm])
                    rin = tpool.tile([128, 1], dt.float32, tag="rin", name="rin")
                    nc.vector.reciprocal(rin[:pm], nrm[:pm])
                    nc.scalar.mul(chn[m][:pm], cn[:pm], rin[:pm])
                    for k in range(KD):
                        pt = ptr2.tile([128, 128], dt.float32, tag="tr2", name="tr2")
                        nc.tensor.transpose(pt[:, :pm],
                                            chn[m][:pm, k * 128:(k + 1) * 128],
                                            ident[:pm, :pm])
                        nc.scalar.copy(cT[k][:, m * 128:m * 128 + pm], pt[:, :pm])

            # ---------- d2 matmul + exact top-2 (5 DVE passes, host decode) ----------
            i1buf = cpool.tile([128, T], dt.float32)
            isbuf = cpool.tile([128, T], dt.float32)
            m1buf = cpool.tile([128, T], dt.float32)
            m2buf = cpool.tile([128, T], dt.float32)
            with tc.tile_pool(name="pd2", bufs=4, space="PSUM") as pd2:
                for t in range(T):
                    sp1 = pd2.tile([128, C], dt.float32, tag="sp1", name="sp1")
                    for (n0, n1) in NSL:
                        for k in range(KD):
                            nc.tensor.matmul(sp1[:, n0:n1],
                                             fhT[k][:, t * 128:(t + 1) * 128],
                                             cT[k][:, n0:n1],
                                             start=(k == 0), stop=(k == KD - 1))

                    # raw-s domain; masked-max zero-trap needs top2 > 0 which
                    # holds for a max over 1000 near-symmetric cosine sims
                    ssb = wpool.tile([128, C], dt.float32, tag="ssb", name="ssb")
                    nc.scalar.copy(ssb[:], sp1[:])
                    m1p = m1buf[:, t:t + 1]
                    nc.vector.tensor_reduce(m1p, sp1[:], AX.X, ALU.max)
                    scr1 = wpool.tile([128, C], dt.bfloat16, tag="scr", name="scr1", bufs=4)
                    nc.vector.scalar_tensor_tensor(scr1[:], sp1[:], m1p, iota_rev[:],
                                                   ALU.is_ge, ALU.mult,
                                                   accum_out=i1buf[:, t:t + 1])
                    mk = wpool.tile([128, C], dt.float32, tag="mk", name="mk", bufs=2)
                    nc.vector.scalar_tensor_tensor(mk[:], ssb[:], m1p, ssb[:],
                                                   ALU.is_lt, ALU.mult)
                    m2p = m2buf[:, t:t + 1]
                    nc.vector.tensor_reduce(m2p, mk[:], AX.X, ALU.max)
                    scr3 = wpool.tile([128, C], dt.bfloat16, tag="scr", name="scr3", bufs=4)
                    nc.vector.scalar_tensor_tensor(scr3[:], ssb[:], m2p, iota_rev[:],
                                                   ALU.is_ge, ALU.mult,
                                                   accum_out=isbuf[:, t:t + 1])

            nc.sync.dma_start(o_i1[:], i1buf[:])
            nc.sync.dma_start(o_is[:], isbuf[:])
            nc.sync.dma_start(o_m1[:], m1buf[:])
            nc.sync.dma_start(o_m2[:], m2buf[:])

    nc.compile()
    return nc


_prog_cache = {}


def _get_program(b_local, n_cores):
    key = (b_local, n_cores)
    if key not in _prog_cache:
        _prog_cache[key] = build_program(b_local, n_cores)
    return _prog_cache[key]


LAST_RESULTS = None  # BassKernelResults of the last run (for test harness)


def kernel(feature_source, center_mem, source_mem, class2center, interdist,
           label_source, index_source, n_cores=N_CORES, trace=False,
           _runner=None):
    feature_source = np.asarray(feature_source, dtype=np.float32)
    center_mem = np.asarray(center_mem, dtype=np.float32)
    source_mem = np.asarray(source_mem, dtype=np.float32)
    class2center = np.asarray(class2center, dtype=np.float32)
    interdist = np.asarray(interdist, dtype=np.float32)
    label_source = np.asarray(label_source, dtype=np.int32)
    index_source = np.asarray(index_source, dtype=np.int32)

    B = feature_source.shape[0]
    b_local = B // n_cores
    T = b_local // 128

    nc = _get_program(b_local, n_cores)

    in_maps = []
    for k in range(n_cores):
        sl = slice(k * b_local, (k + 1) * b_local)
        lab_k = label_source[sl].astype(np.float32).reshape(T, 128).T.copy()
        in_maps.append({
            "feat": feature_source[sl],
            "cenm": center_mem,
            "labf": lab_k,
            "oldrows": np.ascontiguousarray(source_mem[index_source[sl]]),
        })

    global LAST_RESULTS
    if _runner is not None:
        outs = _runner(nc, in_maps)
    else:
        res = run_bass_kernel_spmd(nc, in_maps, core_ids=list(range(n_cores)),
                                   trace=trace,
                                   trace_cores=list(range(n_cores)) if trace else None)
        LAST_RESULTS = res
        outs = res.results

    # ---------- host assembly ----------
    center_new = outs[0]["o_center"]

    source_mem_new = source_mem.copy()
    new_rows = np.concatenate(
        [outs[k]["o_newrows"] for k in range(n_cores)], axis=0)
    source_mem_new[index_source] = new_rows

    counts = np.bincount(label_source, minlength=C).astype(np.float32)
    class2center_new = class2center.copy()
    class2center_new[:, 0] += counts
    class2center_new[:, 1] += 0.5 * counts + outs[0]["o_cdist"][:, 0]

    i1 = np.concatenate([outs[k]["o_i1"].T.reshape(-1) for k in range(n_cores)])
    isum = np.concatenate([outs[k]["o_is"].T.reshape(-1) for k in range(n_cores)])
    m1 = np.concatenate([outs[k]["o_m1"].T.reshape(-1) for k in range(n_cores)])
    m2 = np.concatenate([outs[k]["o_m2"].T.reshape(-1) for k in range(n_cores)])
    top1 = np.rint(float(C - 1) - i1).astype(np.int64)
    top2 = np.rint(float(C - 1) - (isum - i1)).astype(np.int64)
    inter = 0.5 * (m1 - m2)

    interdist_new = interdist.copy()
    flat = interdist_new.reshape(C * C, 2)
    pair = top1.astype(np.int64) * C + top2.astype(np.int64)
    np.add.at(flat[:, 0], pair, 1.0)
    np.add.at(flat[:, 1], pair, inter.astype(np.float64))

    return (center_new, source_mem_new, class2center_new, interdist_new)
